# revision 1
# baseline (speedup 1.0000x reference)
"""Trainium2 Bass kernel for causal attention (B=4, S=4096, D_IN=1024, D_OUT=64).

Sharding: 2 cores per batch. Query rows are interleaved at 128-row tile
granularity (even core takes global q-tiles {0,2,...}, odd takes {1,3,...})
so the causal work per local tile index is core-independent (SPMD-safe) and
balanced. K^T/V^T are all-gathered pairwise (4 chunked AllGathers, pipelined
with compute). All matmuls in bf16 with f32 PSUM accumulation.

Per-core program (local q-tiles t=0..15, global qt = 2t + parity):
  - load x shard (bf16, cast on host), transpose via identity matmuls -> x^T
  - Q^T and K^T/V^T projections (Wq/Wk interleaved into one M=128 stationary)
  - chunked AllGather of [K^T; V^T]; V^T transposed to V-natural tiles with
    an embedded ones column (rowsum via the AV matmul)
  - attention: S^T = K^T.T @ Q^T per 128-sk-tile, exp on ScalarE (scale=1/8
    folded in), causal masks as per-core multiplicative data on the last two
    sk tiles of each q-tile, out^T[65, sq] += [V|1].T @ P^T in PSUM
  - out^T (64 rows AV + 1 row rowsum) to DRAM; host divides, transposes,
    and reassembles the interleaved q-tiles.
"""

import numpy as np
import ml_dtypes

import concourse.bass as bass
import concourse.tile as tile
from concourse import bacc, mybir
from concourse.bass_utils import run_bass_kernel_spmd

B, S, D_IN, D_OUT = 4, 4096, 1024, 64
N_CORES = 8
NCHUNK = 4             # processing chunks (4 row-tiles each)
SCALE = 1.0 / 8.0      # 1/sqrt(D_OUT)
BF16 = mybir.dt.bfloat16
F32 = mybir.dt.float32

REPLICA_GROUPS = [[0, 1], [2, 3], [4, 5], [6, 7]]

ATTENTION_ON = True


def build_kernel(nc, tc, xs, wt, msk, ident, vinit, outT, sim_mode=False):
    from contextlib import ExitStack
    ctx = ExitStack()
    const_pool = ctx.enter_context(tc.tile_pool(name="const", bufs=1))
    ident_sb = const_pool.tile([128, 128], BF16, name="ident_sb")
    msk_sb = const_pool.tile([128, 256], BF16, name="msk_sb")
    # Wq/Wk interleaved per k-tile -> one M=128 stationary per k (single PSUM group)
    wqk_sb = const_pool.tile([128, 1024], BF16, name="wqk_sb")  # cols 128k: [wq_k | wk_k]
    wv_sb = const_pool.tile([128, 512], BF16, name="wv_sb")     # cols 64k: wv_k

    big_pool = ctx.enter_context(tc.tile_pool(name="big", bufs=1))
    qT = big_pool.tile([64, 2048], BF16, name="qT")
    kvT = big_pool.tile([128, 2048], BF16, name="kvT")        # rows 0:64 K^T own, 64:128 V^T own
    kT_ev = big_pool.tile([64, 2048], BF16, name="kT_ev")     # K^T of even global u, cols 128*(u//2)
    kT_od = big_pool.tile([64, 2048], BF16, name="kT_od")
    vT_ev = big_pool.tile([64, 2048], BF16, name="vT_ev")
    vT_od = big_pool.tile([64, 2048], BF16, name="vT_od")
    v_all = big_pool.tile([128, 32 * 66], BF16, name="v_all")  # V natural tiles + ones col
    xTall = big_pool.tile([128, 8 * 2048], BF16, name="xTall")  # x^T: k-tile k at cols 2048k
    outT_sb = big_pool.tile([65, 2048], F32, name="outT_sb")

    pt_pool = ctx.enter_context(tc.tile_pool(name="pt", bufs=4))

    ps1_pool = ctx.enter_context(tc.tile_pool(name="ps1", bufs=2, space="PSUM"))
    ps2_pool = ctx.enter_context(tc.tile_pool(name="ps2", bufs=2, space="PSUM"))
    acc_pool = ctx.enter_context(tc.tile_pool(name="accp", bufs=2, space="PSUM"))

    dram_pool = ctx.enter_context(tc.tile_pool(name="dram", bufs=1, space="DRAM"))

    # constants
    nc.sync.dma_start(ident_sb[:], ident[:])
    nc.sync.dma_start(msk_sb[:], msk[:])
    wqk_view = wqk_sb[:].rearrange("p (k t o) -> p k t o", k=8, t=2)
    nc.sync.dma_start(wqk_view[:, :, 0, :], wt[0].rearrange("(k p) o -> p k o", p=128))
    nc.sync.dma_start(wqk_view[:, :, 1, :], wt[1].rearrange("(k p) o -> p k o", p=128))
    nc.sync.dma_start(
        wv_sb[:].rearrange("p (k o) -> p k o", k=8),
        wt[2].rearrange("(k p) o -> p k o", p=128),
    )
    nc.sync.dma_start(v_all[:], vinit[:])

    v_view = v_all[:].rearrange("p (u e) -> p u e", e=66)
    ident64 = ident_sb[0:64, 0:64]

    def ksl(u):
        t = kT_ev if u % 2 == 0 else kT_od
        return t[:, 128 * (u // 2) : 128 * (u // 2) + 128]

    # all x^T transposes up front: 8 full-column xbar DMAs (one mode region)
    for k in range(8):
        nc.sync.dma_start(
            xTall[:, 2048 * k : 2048 * (k + 1)],
            xs[:, 128 * k : 128 * (k + 1)],
            transpose=True,
        )

    def chunk_phase(c):
        """Project + AllGather chunk c (pairs 4c..4c+3)."""
        # ---- projections for this chunk's 512 s-columns
        qk_ps = ps1_pool.tile([128, 512], F32, tag="ps1")
        v_ps = ps1_pool.tile([64, 512], F32, tag="ps1")
        for k in range(8):
            rhs = xTall[:, 2048 * k + 512 * c : 2048 * k + 512 * (c + 1)]
            nc.tensor.matmul(qk_ps[:, :], wqk_sb[:, 128 * k : 128 * (k + 1)], rhs,
                             start=(k == 0), stop=(k == 7))
            nc.tensor.matmul(v_ps[:, :], wv_sb[:, 64 * k : 64 * k + 64], rhs,
                             start=(k == 0), stop=(k == 7))
        cols = slice(512 * c, 512 * (c + 1))
        nc.vector.tensor_copy(qT[:, cols], qk_ps[0:64, :])
        nc.vector.tensor_copy(kvT[0:64, cols], qk_ps[64:128, :])
        nc.vector.tensor_copy(kvT[64:128, cols], v_ps[:, :])

        # ---- chunked AllGather of [K^T; V^T]
        cc_in = dram_pool.tile([128, 512], BF16, name=f"cc_in{c}", tag=f"cc_in{c}")
        cc_out = dram_pool.tile([256, 512], BF16, name=f"cc_out{c}", tag=f"cc_out{c}")
        nc.sync.dma_start(cc_in[:], kvT[:, cols])
        if sim_mode:
            nc.sync.dma_start(cc_out[0:128, :], cc_in[:])
            nc.sync.dma_start(cc_out[128:256, :], cc_in[:])
        else:
            nc.gpsimd.collective_compute(
                "AllGather", mybir.AluOpType.bypass,
                replica_groups=REPLICA_GROUPS,
                ins=[cc_in[:].opt()], outs=[cc_out[:].opt()],
            )
        nc.sync.dma_start(kT_ev[:, cols], cc_out[0:64, :])
        nc.sync.dma_start(vT_ev[:, cols], cc_out[64:128, :])
        nc.sync.dma_start(kT_od[:, cols], cc_out[128:192, :])
        nc.sync.dma_start(vT_od[:, cols], cc_out[192:256, :])

        # ---- V natural tiles (u = 2p, 2p+1 for pairs p in this chunk)
        for phalf in range(2):  # 4 u-tiles of 64 cols per psum tile
            vt_ps = ps1_pool.tile([128, 512], F32, tag="ps1")
            u_base = 2 * (4 * c + 2 * phalf)
            for pp in range(2):
                p = 4 * c + 2 * phalf + pp
                sl = slice(128 * p, 128 * (p + 1))
                off = 128 * pp
                nc.tensor.matmul(vt_ps[:, off : off + 64], vT_ev[:, sl],
                                 ident64, start=True, stop=True)
                nc.tensor.matmul(vt_ps[:, off + 64 : off + 128], vT_od[:, sl],
                                 ident64, start=True, stop=True)
            nc.vector.tensor_copy(
                v_view[:, u_base : u_base + 4, 0:64],
                vt_ps[:, 0:256].rearrange("p (u e) -> p u e", e=64),
            )

    def attention_group(g):
        """q-tiles 4g..4g+3 (sq cols 512g..512g+512)."""
        acc = acc_pool.tile([65, 512], F32, tag="acc")
        qcols = slice(512 * g, 512 * (g + 1))
        # bulk: pairs p < 4g, full 512-wide
        for p in range(4 * g):
            sb = ps2_pool.tile([128, 1024], F32, tag="sb")
            nc.tensor.matmul(sb[:, 0:512], ksl(2 * p), qT[:, qcols],
                             start=True, stop=True)
            nc.tensor.matmul(sb[:, 512:1024], ksl(2 * p + 1), qT[:, qcols],
                             start=True, stop=True)
            pt = pt_pool.tile([128, 1024], BF16, tag="pt")
            nc.scalar.activation(pt[:], sb[:], mybir.ActivationFunctionType.Exp, scale=SCALE)
            nc.tensor.matmul(acc[:], v_view[:, 2 * p, 0:65], pt[:, 0:512],
                             start=(p == 0), stop=False)
            nc.tensor.matmul(acc[:], v_view[:, 2 * p + 1, 0:65], pt[:, 512:1024],
                             start=False, stop=False)
        # tail: per q-tile a, k-tiles u in [8g, 8g+2a+2)
        for a in range(4):
            t_loc = 4 * g + a
            qc = slice(128 * t_loc, 128 * (t_loc + 1))
            width = (2 * a + 2) * 128
            stb = ps2_pool.tile([128, 1024], F32, tag="sb")
            for i in range(2 * a + 2):
                u = 8 * g + i
                nc.tensor.matmul(stb[:, 128 * i : 128 * (i + 1)], ksl(u), qT[:, qc],
                                 start=True, stop=True)
            ptt = pt_pool.tile([128, 1024], BF16, tag="pt")
            nc.scalar.activation(ptt[:, 0:width], stb[:, 0:width],
                                 mybir.ActivationFunctionType.Exp, scale=SCALE)
            nc.vector.tensor_mul(ptt[:, 256 * a : 256 * a + 128],
                                 ptt[:, 256 * a : 256 * a + 128], msk_sb[:, 0:128])
            nc.vector.tensor_mul(ptt[:, 256 * a + 128 : 256 * a + 256],
                                 ptt[:, 256 * a + 128 : 256 * a + 256], msk_sb[:, 128:256])
            for i in range(2 * a + 2):
                u = 8 * g + i
                if g == 0:
                    st, sp = (i == 0), (i == 2 * a + 1)
                else:
                    st, sp = False, (a == 3 and i == 2 * a + 1)
                nc.tensor.matmul(acc[0:65, 128 * a : 128 * (a + 1)],
                                 v_view[:, u, 0:65], ptt[:, 128 * i : 128 * (i + 1)],
                                 start=st, stop=sp)
        nc.vector.tensor_copy(outT_sb[:, qcols], acc[:])
        nc.sync.dma_start(outT[:, qcols], outT_sb[:, qcols])

    def attention_dummy(g):
        qcols = slice(512 * g, 512 * (g + 1))
        nc.vector.tensor_copy(outT_sb[0:64, qcols], kT_ev[:, qcols])
        nc.vector.tensor_copy(outT_sb[64:65, qcols], v_all[0:1, 0:512])
        nc.sync.dma_start(outT[:, qcols], outT_sb[:, qcols])

    for c in range(NCHUNK):
        chunk_phase(c)
        if ATTENTION_ON:
            attention_group(c)
        else:
            attention_dummy(c)

    ctx.close()


def build_nc(sim_mode=False):
    nc = bacc.Bacc("TRN2", target_bir_lowering=False, debug=False,
                   num_devices=1 if sim_mode else N_CORES)
    xs = nc.dram_tensor("xs", [2048, 1024], BF16, kind="ExternalInput").ap()
    wt = nc.dram_tensor("wt", [3, 1024, 64], BF16, kind="ExternalInput").ap()
    msk = nc.dram_tensor("msk", [128, 256], BF16, kind="ExternalInput").ap()
    ident = nc.dram_tensor("ident", [128, 128], BF16, kind="ExternalInput").ap()
    vinit = nc.dram_tensor("vinit", [128, 32 * 66], BF16, kind="ExternalInput").ap()
    outT = nc.dram_tensor("outT", [65, 2048], F32, kind="ExternalOutput").ap()
    with tile.TileContext(nc) as tc:
        build_kernel(nc, tc, xs, wt, msk, ident, vinit, outT, sim_mode=sim_mode)
    nc.compile()
    return nc


_NC_CACHE = None


def get_nc():
    global _NC_CACHE
    if _NC_CACHE is None:
        _NC_CACHE = build_nc()
    return _NC_CACHE


def make_in_maps(x, Wq, Wk, Wv):
    bf = ml_dtypes.bfloat16
    wt = np.ascontiguousarray(
        np.stack([Wq.T, Wk.T, Wv.T]).astype(bf))  # [3, 1024, 64]
    ident = np.eye(128, dtype=bf)
    vinit = np.ones((128, 32 * 66), dtype=bf)
    tri = np.tril(np.ones((128, 128), dtype=np.float32)).T  # [i, q] = 1 if i <= q
    tri = np.ascontiguousarray(tri.astype(bf))
    ones = np.ones((128, 128), dtype=bf)
    zeros = np.zeros((128, 128), dtype=bf)
    msk_even = np.ascontiguousarray(np.concatenate([tri, zeros], axis=1))
    msk_odd = np.ascontiguousarray(np.concatenate([ones, tri], axis=1))
    in_maps = []
    for c in range(N_CORES):
        b, j = c // 2, c % 2
        xsh = np.ascontiguousarray(
            x[b].reshape(32, 128, D_IN)[j::2].reshape(2048, D_IN).astype(bf))
        in_maps.append({
            "xs": xsh,
            "wt": wt,
            "msk": msk_even if j == 0 else msk_odd,
            "ident": ident,
            "vinit": vinit,
        })
    return in_maps


def assemble_output(results):
    out = np.empty((B, S, D_OUT), dtype=np.float32)
    for c in range(N_CORES):
        b, j = c // 2, c % 2
        oT = results[c]["outT"].astype(np.float32)  # [65, 2048]
        o = (oT[:64] / oT[64:65]).T                 # [2048, 64]
        out[b].reshape(32, 128, D_OUT)[j::2] = o.reshape(16, 128, D_OUT)
    return out


def kernel(x, Wq, Wk, Wv):
    nc = get_nc()
    in_maps = make_in_maps(np.asarray(x), np.asarray(Wq), np.asarray(Wk), np.asarray(Wv))
    res = run_bass_kernel_spmd(nc, in_maps, core_ids=list(range(N_CORES)))
    return assemble_output(res.results)



# revision 2
# speedup vs baseline: 15.0628x; 15.0628x over previous
"""Trainium2 Bass kernel for causal attention (B=4, S=4096, D_IN=1024, D_OUT=64).

Sharding: 2 cores per batch. Query rows are interleaved at 128-row tile
granularity (even core takes global q-tiles {0,2,...}, odd takes {1,3,...})
so the causal work per local tile index is core-independent (SPMD-safe) and
balanced. K^T/V^T are all-gathered pairwise (4 chunked AllGathers, pipelined
with compute). All matmuls in bf16 with f32 PSUM accumulation.

Per-core program (local q-tiles t=0..15, global qt = 2t + parity):
  - load x shard (bf16, cast on host), transpose via identity matmuls -> x^T
  - Q^T and K^T/V^T projections (Wq/Wk interleaved into one M=128 stationary)
  - chunked AllGather of [K^T; V^T]; V^T transposed to V-natural tiles with
    an embedded ones column (rowsum via the AV matmul)
  - attention: S^T = K^T.T @ Q^T per 128-sk-tile, exp on ScalarE (scale=1/8
    folded in), causal masks as per-core multiplicative data on the last two
    sk tiles of each q-tile, out^T[65, sq] += [V|1].T @ P^T in PSUM
  - out^T (64 rows AV + 1 row rowsum) to DRAM; host divides, transposes,
    and reassembles the interleaved q-tiles.
"""

import numpy as np
import ml_dtypes

import concourse.bass as bass
import concourse.tile as tile
from concourse import bacc, mybir
from concourse.bass_utils import run_bass_kernel_spmd

B, S, D_IN, D_OUT = 4, 4096, 1024, 64
N_CORES = 8
NCHUNK = 4             # processing chunks (4 row-tiles each)
SCALE = 1.0 / 8.0      # 1/sqrt(D_OUT)
BF16 = mybir.dt.bfloat16
F32 = mybir.dt.float32

REPLICA_GROUPS = [[0, 1], [2, 3], [4, 5], [6, 7]]

ATTENTION_ON = True


def build_kernel(nc, tc, xs, wt, msk, ident, vinit, outT, sim_mode=False):
    from contextlib import ExitStack
    ctx = ExitStack()
    const_pool = ctx.enter_context(tc.tile_pool(name="const", bufs=1))
    ident_sb = const_pool.tile([128, 128], BF16, name="ident_sb")
    msk_sb = const_pool.tile([128, 256], BF16, name="msk_sb")
    # Wq/Wk interleaved per k-tile -> one M=128 stationary per k (single PSUM group)
    wqk_sb = const_pool.tile([128, 1024], BF16, name="wqk_sb")  # cols 128k: [wq_k | wk_k]
    wv_sb = const_pool.tile([128, 512], BF16, name="wv_sb")     # cols 64k: wv_k

    big_pool = ctx.enter_context(tc.tile_pool(name="big", bufs=1))
    qT = big_pool.tile([64, 2048], BF16, name="qT")
    kvT = big_pool.tile([128, 2048], BF16, name="kvT")        # rows 0:64 K^T own, 64:128 V^T own
    kT_ev = big_pool.tile([64, 2048], BF16, name="kT_ev")     # K^T of even global u, cols 128*(u//2)
    kT_od = big_pool.tile([64, 2048], BF16, name="kT_od")
    vT_ev = big_pool.tile([64, 2048], BF16, name="vT_ev")
    vT_od = big_pool.tile([64, 2048], BF16, name="vT_od")
    v_all = big_pool.tile([128, 32 * 66], BF16, name="v_all")  # V natural tiles + ones col
    xTall = big_pool.tile([128, 8 * 2048], BF16, name="xTall")  # x^T: k-tile k at cols 2048k
    outT_sb = big_pool.tile([65, 2048], F32, name="outT_sb")

    pt_pool = ctx.enter_context(tc.tile_pool(name="pt", bufs=4))

    ps1_pool = ctx.enter_context(tc.tile_pool(name="ps1", bufs=2, space="PSUM"))
    ps2_pool = ctx.enter_context(tc.tile_pool(name="ps2", bufs=2, space="PSUM"))
    acc_pool = ctx.enter_context(tc.tile_pool(name="accp", bufs=2, space="PSUM"))

    dram_pool = ctx.enter_context(tc.tile_pool(name="dram", bufs=1, space="DRAM"))

    # constants
    nc.sync.dma_start(ident_sb[:], ident[:])
    nc.sync.dma_start(msk_sb[:], msk[:])
    wqk_view = wqk_sb[:].rearrange("p (k t o) -> p k t o", k=8, t=2)
    nc.sync.dma_start(wqk_view[:, :, 0, :], wt[0].rearrange("(k p) o -> p k o", p=128))
    nc.sync.dma_start(wqk_view[:, :, 1, :], wt[1].rearrange("(k p) o -> p k o", p=128))
    nc.sync.dma_start(
        wv_sb[:].rearrange("p (k o) -> p k o", k=8),
        wt[2].rearrange("(k p) o -> p k o", p=128),
    )
    nc.sync.dma_start(v_all[:], vinit[:])

    v_view = v_all[:].rearrange("p (u e) -> p u e", e=66)
    ident64 = ident_sb[0:64, 0:64]

    def ksl(u):
        t = kT_ev if u % 2 == 0 else kT_od
        return t[:, 128 * (u // 2) : 128 * (u // 2) + 128]

    # all x^T transposes up front: 8 full-column xbar DMAs (one mode region)
    for k in range(8):
        nc.sync.dma_start(
            xTall[:, 2048 * k : 2048 * (k + 1)],
            xs[:, 128 * k : 128 * (k + 1)],
            transpose=True,
        )

    def chunk_phase(c):
        """Project + AllGather chunk c (pairs 4c..4c+3)."""
        # ---- projections for this chunk's 512 s-columns
        qk_ps = ps1_pool.tile([128, 512], F32, tag="ps1")
        v_ps = ps1_pool.tile([64, 512], F32, tag="ps1")
        for k in range(8):
            rhs = xTall[:, 2048 * k + 512 * c : 2048 * k + 512 * (c + 1)]
            nc.tensor.matmul(qk_ps[:, :], wqk_sb[:, 128 * k : 128 * (k + 1)], rhs,
                             start=(k == 0), stop=(k == 7))
            nc.tensor.matmul(v_ps[:, :], wv_sb[:, 64 * k : 64 * k + 64], rhs,
                             start=(k == 0), stop=(k == 7))
        cols = slice(512 * c, 512 * (c + 1))
        nc.vector.tensor_copy(qT[:, cols], qk_ps[0:64, :])
        nc.vector.tensor_copy(kvT[0:64, cols], qk_ps[64:128, :])
        nc.vector.tensor_copy(kvT[64:128, cols], v_ps[:, :])

        # ---- chunked AllGather of [K^T; V^T]
        cc_in = dram_pool.tile([128, 512], BF16, name=f"cc_in{c}", tag=f"cc_in{c}")
        cc_out = dram_pool.tile([256, 512], BF16, name=f"cc_out{c}", tag=f"cc_out{c}")
        nc.sync.dma_start(cc_in[:], kvT[:, cols])
        if sim_mode:
            nc.sync.dma_start(cc_out[0:128, :], cc_in[:])
            nc.sync.dma_start(cc_out[128:256, :], cc_in[:])
        else:
            nc.gpsimd.collective_compute(
                "AllGather", mybir.AluOpType.bypass,
                replica_groups=REPLICA_GROUPS,
                ins=[cc_in[:].opt()], outs=[cc_out[:].opt()],
            )
        nc.sync.dma_start(kT_ev[:, cols], cc_out[0:64, :])
        nc.sync.dma_start(vT_ev[:, cols], cc_out[64:128, :])
        nc.sync.dma_start(kT_od[:, cols], cc_out[128:192, :])
        nc.sync.dma_start(vT_od[:, cols], cc_out[192:256, :])

        # ---- V natural tiles (u = 2p, 2p+1 for pairs p in this chunk)
        for phalf in range(2):  # 4 u-tiles of 64 cols per psum tile
            vt_ps = ps1_pool.tile([128, 512], F32, tag="ps1")
            u_base = 2 * (4 * c + 2 * phalf)
            for pp in range(2):
                p = 4 * c + 2 * phalf + pp
                sl = slice(128 * p, 128 * (p + 1))
                off = 128 * pp
                nc.tensor.matmul(vt_ps[:, off : off + 64], vT_ev[:, sl],
                                 ident64, start=True, stop=True)
                nc.tensor.matmul(vt_ps[:, off + 64 : off + 128], vT_od[:, sl],
                                 ident64, start=True, stop=True)
            nc.vector.tensor_copy(
                v_view[:, u_base : u_base + 4, 0:64],
                vt_ps[:, 0:256].rearrange("p (u e) -> p u e", e=64),
            )

    def attention_group(g):
        """q-tiles 4g..4g+3 (sq cols 512g..512g+512)."""
        acc = acc_pool.tile([65, 512], F32, tag="acc")
        qcols = slice(512 * g, 512 * (g + 1))
        # bulk: pairs p < 4g, full 512-wide
        for p in range(4 * g):
            sb = ps2_pool.tile([128, 1024], F32, tag="sb")
            nc.tensor.matmul(sb[:, 0:512], ksl(2 * p), qT[:, qcols],
                             start=True, stop=True)
            nc.tensor.matmul(sb[:, 512:1024], ksl(2 * p + 1), qT[:, qcols],
                             start=True, stop=True)
            pt = pt_pool.tile([128, 1024], BF16, tag="pt")
            nc.scalar.activation(pt[:], sb[:], mybir.ActivationFunctionType.Exp, scale=SCALE)
            nc.tensor.matmul(acc[:], v_view[:, 2 * p, 0:65], pt[:, 0:512],
                             start=(p == 0), stop=False)
            nc.tensor.matmul(acc[:], v_view[:, 2 * p + 1, 0:65], pt[:, 512:1024],
                             start=False, stop=False)
        # tail: per q-tile a, k-tiles u in [8g, 8g+2a+2)
        for a in range(4):
            t_loc = 4 * g + a
            qc = slice(128 * t_loc, 128 * (t_loc + 1))
            width = (2 * a + 2) * 128
            stb = ps2_pool.tile([128, 1024], F32, tag="sb")
            for i in range(2 * a + 2):
                u = 8 * g + i
                nc.tensor.matmul(stb[:, 128 * i : 128 * (i + 1)], ksl(u), qT[:, qc],
                                 start=True, stop=True)
            ptt = pt_pool.tile([128, 1024], BF16, tag="pt")
            nc.scalar.activation(ptt[:, 0:width], stb[:, 0:width],
                                 mybir.ActivationFunctionType.Exp, scale=SCALE)
            nc.vector.tensor_mul(ptt[:, 256 * a : 256 * a + 128],
                                 ptt[:, 256 * a : 256 * a + 128], msk_sb[:, 0:128])
            nc.vector.tensor_mul(ptt[:, 256 * a + 128 : 256 * a + 256],
                                 ptt[:, 256 * a + 128 : 256 * a + 256], msk_sb[:, 128:256])
            for i in range(2 * a + 2):
                u = 8 * g + i
                if g == 0:
                    st, sp = (i == 0), (i == 2 * a + 1)
                else:
                    st, sp = False, (a == 3 and i == 2 * a + 1)
                nc.tensor.matmul(acc[0:65, 128 * a : 128 * (a + 1)],
                                 v_view[:, u, 0:65], ptt[:, 128 * i : 128 * (i + 1)],
                                 start=st, stop=sp)
        nc.vector.tensor_copy(outT_sb[:, qcols], acc[:])
        nc.sync.dma_start(outT[:, qcols], outT_sb[:, qcols])

    def attention_dummy(g):
        qcols = slice(512 * g, 512 * (g + 1))
        nc.vector.tensor_copy(outT_sb[0:64, qcols], kT_ev[:, qcols])
        nc.vector.tensor_copy(outT_sb[64:65, qcols], v_all[0:1, 0:512])
        nc.sync.dma_start(outT[:, qcols], outT_sb[:, qcols])

    for c in range(NCHUNK):
        chunk_phase(c)
        if ATTENTION_ON:
            attention_group(c)
        else:
            attention_dummy(c)

    ctx.close()


def declare_io(nc):
    xs = nc.dram_tensor("xs", [2048, 1024], BF16, kind="ExternalInput").ap()
    wt = nc.dram_tensor("wt", [3, 1024, 64], BF16, kind="ExternalInput").ap()
    msk = nc.dram_tensor("msk", [128, 256], BF16, kind="ExternalInput").ap()
    ident = nc.dram_tensor("ident", [128, 128], BF16, kind="ExternalInput").ap()
    vinit = nc.dram_tensor("vinit", [128, 32 * 66], BF16, kind="ExternalInput").ap()
    outT = nc.dram_tensor("outT", [65, 2048], F32, kind="ExternalOutput").ap()
    return xs, wt, msk, ident, vinit, outT


def build_nc(sim_mode=False):
    nc = bacc.Bacc("TRN2", target_bir_lowering=False, debug=False,
                   num_devices=1 if sim_mode else N_CORES)
    handles = declare_io(nc)
    with tile.TileContext(nc) as tc:
        build_kernel(nc, tc, *handles, sim_mode=sim_mode)
    nc.compile()
    return nc


_NC_CACHE = None


def get_nc():
    global _NC_CACHE
    if _NC_CACHE is None:
        _NC_CACHE = build_nc()
    return _NC_CACHE


def make_in_maps(x, Wq, Wk, Wv):
    bf = ml_dtypes.bfloat16
    wt = np.ascontiguousarray(
        np.stack([Wq.T, Wk.T, Wv.T]).astype(bf))  # [3, 1024, 64]
    ident = np.eye(128, dtype=bf)
    vinit = np.ones((128, 32 * 66), dtype=bf)
    tri = np.tril(np.ones((128, 128), dtype=np.float32)).T  # [i, q] = 1 if i <= q
    tri = np.ascontiguousarray(tri.astype(bf))
    ones = np.ones((128, 128), dtype=bf)
    zeros = np.zeros((128, 128), dtype=bf)
    msk_even = np.ascontiguousarray(np.concatenate([tri, zeros], axis=1))
    msk_odd = np.ascontiguousarray(np.concatenate([ones, tri], axis=1))
    in_maps = []
    for c in range(N_CORES):
        b, j = c // 2, c % 2
        xsh = np.ascontiguousarray(
            x[b].reshape(32, 128, D_IN)[j::2].reshape(2048, D_IN).astype(bf))
        in_maps.append({
            "xs": xsh,
            "wt": wt,
            "msk": msk_even if j == 0 else msk_odd,
            "ident": ident,
            "vinit": vinit,
        })
    return in_maps


def assemble_output(results):
    out = np.empty((B, S, D_OUT), dtype=np.float32)
    for c in range(N_CORES):
        b, j = c // 2, c % 2
        oT = results[c]["outT"].astype(np.float32)  # [65, 2048]
        o = (oT[:64] / oT[64:65]).T                 # [2048, 64]
        out[b].reshape(32, 128, D_OUT)[j::2] = o.reshape(16, 128, D_OUT)
    return out


def kernel(x, Wq, Wk, Wv):
    nc = get_nc()
    in_maps = make_in_maps(np.asarray(x), np.asarray(Wq), np.asarray(Wk), np.asarray(Wv))
    res = run_bass_kernel_spmd(nc, in_maps, core_ids=list(range(N_CORES)))
    return assemble_output(res.results)

